# revision 1
# baseline (speedup 1.0000x reference)
"""Trainium2 Bass kernel for nn_NodeBlock (GNN message passing).

Pipeline: segment_sum of edge features onto destination nodes, concat with
node features, 3-layer MLP, LayerNorm.

Sharding: nodes are range-sharded across the 8 cores (12800 nodes/core, 100
blocks of 128). On the host, edges are bucketed by destination-node block
(a shard of the edge list per core, padded per block-position to a uniform
tile count Kb across cores), so each core streams only the edge rows it
needs, contiguously. Edge values are split hi/lo into two fp16 tensors
(hi = fp16(x), lo = fp16(x - hi), ~2e-7 combined relative error) so the
PE runs at full 1-cycle/row rate instead of fp32's 4 cycles/row.

On device, per 128-node block: the segment sum is a pair of one-hot fp16
matmuls accumulated in PSUM (aggrT[f, j] = sum_e hi[e, f] * oh[e, j] +
lo[e, f] * oh[e, j], oh = (col_local == j)), followed by the f32 MLP in
feature-major layout and a PE transpose + LayerNorm.
"""

import sys

sys.path.insert(0, "/opt/trn_rl_repo")

import numpy as np

N_CORES = 8
NUM_NODES = 100000
D = 128            # node/edge feature dim
P = 128            # partitions
BLK = 128          # nodes per block
BLOCKS_PER_CORE = 100
NODES_PER_CORE = BLK * BLOCKS_PER_CORE   # 12800
TOTAL_BLOCKS = N_CORES * BLOCKS_PER_CORE  # 800
EPS = 1e-5

_nc_cache = {}
last_run_info = {}


TUNE = {"ebufs": 4, "ohbufs": 3, "sbufs": 4, "agbufs": 3, "mlpbufs": 3,
        "oh_mode": "ts", "dma_split": True, "edge_pair": False,
        "only": None}


def _build_nc(kb, loop_iters=None):
    """kb: tuple of per-block-position edge-tile counts (len 100)."""
    import contextlib
    import concourse.bacc as bacc
    import concourse.tile as tile
    import concourse.mybir as mybir
    from concourse.masks import make_identity

    dt = mybir.dt
    f32 = dt.float32
    f16 = dt.float16
    kb = list(kb)
    kmax = max(kb)
    tot_e = sum(k * 256 for k in kb)   # per-partition fp16 elems (hi+lo)
    tot_c = sum(kb)

    nc = bacc.Bacc("TRN2", target_bir_lowering=False, debug=False,
                   name="nodeblock")

    edges = nc.dram_tensor("edges", [P, tot_e], f16, kind="ExternalInput")
    colf = nc.dram_tensor("colf", [P, tot_c], f16, kind="ExternalInput")
    colf32 = nc.dram_tensor("colf32", [P, tot_c], f32, kind="ExternalInput")
    natT = nc.dram_tensor("natT", [P, NODES_PER_CORE], f32,
                          kind="ExternalInput")
    iota = nc.dram_tensor("iota", [P, kmax, 128], f16, kind="ExternalInput")
    w_in = {}
    for nm in ["w0a", "w0b", "w1", "w2", "gam", "bet"]:
        w_in[nm] = nc.dram_tensor(nm, [128, 128], f32, kind="ExternalInput")
    for nm in ["b0", "b1", "b2"]:
        w_in[nm] = nc.dram_tensor(nm, [128, 1], f32, kind="ExternalInput")
    out = nc.dram_tensor("out", [BLOCKS_PER_CORE, P, D], f32,
                         kind="ExternalOutput")

    with tile.TileContext(nc) as tc:
        with (
            tc.tile_pool(name="const", bufs=1) as cpool,
            tc.tile_pool(name="edge", bufs=TUNE["ebufs"]) as epool,
            tc.tile_pool(name="oh", bufs=TUNE["ohbufs"]) as ohpool,
            tc.tile_pool(name="small", bufs=TUNE["sbufs"]) as spool,
            tc.tile_pool(name="psag", bufs=TUNE["agbufs"],
                         space="PSUM") as psag,
            tc.tile_pool(name="psmlp", bufs=TUNE["mlpbufs"],
                         space="PSUM") as psmlp,
        ):
            cdma = nc.scalar if TUNE["dma_split"] else nc.sync
            colf_s = cpool.tile([P, tot_c], f16, tag="colf", name="colf")
            cdma.dma_start(out=colf_s[:], in_=colf[:])
            colf32_s = cpool.tile([P, tot_c], f32, tag="colf32",
                                  name="colf32")
            cdma.dma_start(out=colf32_s[:], in_=colf32[:])
            natT_s = cpool.tile([P, NODES_PER_CORE], f32, tag="natT",
                                name="natT")
            cdma.dma_start(out=natT_s[:], in_=natT[:])
            iota_s = cpool.tile([P, kmax, 128], f16, tag="iota", name="iota")
            cdma.dma_start(out=iota_s[:], in_=iota[:])
            consts = {}
            for nm, t in w_in.items():
                consts[nm] = cpool.tile(list(t.shape), f32, tag=nm, name=nm)
                cdma.dma_start(out=consts[nm][:], in_=t[:])
            ident = cpool.tile([P, P], f32, tag="ident", name="ident")
            make_identity(nc, ident[:])
            epst = cpool.tile([P, 1], f32, tag="eps", name="eps")
            nc.vector.memset(epst[:], EPS)

            loop_cm = (tc.For_i(0, loop_iters, 1) if loop_iters
                       else contextlib.nullcontext())
            with loop_cm:
                _emit_blocks(nc, tc, kb, epool, ohpool, spool, psag, psmlp,
                             colf_s, colf32_s, natT_s, iota_s, consts, ident,
                             epst, edges, out, mybir)
    nc.finalize()
    return nc


def _emit_blocks(nc, tc, kb, epool, ohpool, spool, psag, psmlp, colf_s,
                 colf32_s, natT_s, iota_s, consts, ident, epst, edges, out,
                 mybir):
    dt = mybir.dt
    f32 = dt.float32
    f16 = dt.float16
    Alu = mybir.AluOpType
    Act = mybir.ActivationFunctionType
    kmax = max(kb)
    e_off = 0
    c_off = 0
    pair = TUNE["edge_pair"]
    only = TUNE["only"]
    do_dma = only in (None, "dma")
    do_oh = only in (None, "dve")
    do_mm = only in (None, "pe")
    do_mlp = only is None
    pair_tile = None
    pair_off = 0
    eblk0 = None
    oh0 = None
    if only == "pe":
        # static operands loaded once; PE work only
        eblk0 = epool.tile([P, 2 * kmax * 128], f16, tag="eblk", name="eblk")
        nc.sync.dma_start(out=eblk0[:], in_=edges[:, :2 * kmax * 128])
        oh0 = ohpool.tile([P, kmax, 128], f16, tag="oh", name="oh")
        csl0 = colf_s[:, 0:kmax].broadcast_to([P, kmax, 128])
        nc.vector.tensor_tensor(out=oh0[:], in0=csl0, in1=iota_s[:],
                                op=Alu.is_equal)
    for b in range(BLOCKS_PER_CORE):
        K = kb[b]
        KE = K * 128
        edma = (nc.sync if (not TUNE["dma_split"] or b % 2 == 0)
                else nc.scalar)
        odma = (nc.scalar if (not TUNE["dma_split"] or b % 2 == 0)
                else nc.sync) if TUNE["dma_split"] else nc.sync
        if do_dma:
            if pair:
                if b % 2 == 0:
                    hi_b = min(b + 1, BLOCKS_PER_CORE - 1)
                    span = sum(2 * kb[i] * 128 for i in range(b, hi_b + 1))
                    pair_tile = epool.tile([P, 4 * kmax * 128], f16,
                                           tag="eblk", name="eblk")
                    edma.dma_start(out=pair_tile[:, :span],
                                   in_=edges[:, e_off:e_off + span])
                    pair_off = 0
                eblk = pair_tile[:, pair_off:pair_off + 2 * KE]
                pair_off += 2 * KE
            else:
                eblk = epool.tile([P, 2 * kmax * 128], f16, tag="eblk",
                                  name="eblk")
                edma.dma_start(out=eblk[:, :2 * KE],
                               in_=edges[:, e_off:e_off + 2 * KE])
        else:
            eblk = eblk0
        e_off += 2 * KE
        if only == "dma":
            c_off += K
            continue
        if only == "pe":
            pag = psag.tile([P, 128], f32, tag="ag", name="ag")
            for k in range(K):
                nc.tensor.matmul(out=pag[:],
                                 lhsT=eblk[:, k * 128:(k + 1) * 128],
                                 rhs=oh0[:, k, :],
                                 start=(k == 0), stop=False)
                nc.tensor.matmul(out=pag[:],
                                 lhsT=eblk[:, KE + k * 128:KE + (k + 1) * 128],
                                 rhs=oh0[:, k, :],
                                 start=False, stop=(k == K - 1))
            c_off += K
            continue

        # one-hot: oh[p, k, j] = (colf[p, c_off + k] == j)
        oh = ohpool.tile([P, kmax, 128], f16, tag="oh", name="oh")
        if TUNE["oh_mode"] == "tt":
            # whole block in one DVE tensor_tensor (runs at 1x: broadcast in0)
            csl = colf_s[:, c_off:c_off + K].broadcast_to([P, K, 128])
            nc.vector.tensor_tensor(out=oh[:, :K, :], in0=csl,
                                    in1=iota_s[:, :K, :], op=Alu.is_equal)
        else:
            # per-tile tensor_scalar (fp16 single-src: 4x mode)
            for k in range(K):
                nc.vector.tensor_scalar(
                    out=oh[:, k, :], in0=iota_s[:, 0, :],
                    scalar1=colf32_s[:, c_off + k:c_off + k + 1],
                    scalar2=None, op0=Alu.is_equal)
        if only == "dve":
            c_off += K
            continue

        # aggrT[f, j] = sum_k sum_e (hi[e,f] + lo[e,f]) * oh[e, k, j]
        pag = psag.tile([P, 128], f32, tag="ag", name="ag")
        for k in range(K):
            nc.tensor.matmul(out=pag[:],
                             lhsT=eblk[:, k * 128:(k + 1) * 128],
                             rhs=oh[:, k, :],
                             start=(k == 0), stop=False)
            nc.tensor.matmul(out=pag[:],
                             lhsT=eblk[:, KE + k * 128:KE + (k + 1) * 128],
                             rhs=oh[:, k, :],
                             start=False, stop=(k == K - 1))
        aggrT = spool.tile([P, 128], f32, tag="aggrT", name="aggrT")
        nc.scalar.copy(aggrT[:], pag[:])

        # h1T = relu(W0a.T @ natT_blk + W0b.T @ aggrT + b0)
        ph1 = psmlp.tile([P, 128], f32, tag="mlp", name="mlp")
        nc.tensor.matmul(out=ph1[:], lhsT=consts["w0a"][:],
                         rhs=natT_s[:, b * 128:(b + 1) * 128],
                         start=True, stop=False)
        nc.tensor.matmul(out=ph1[:], lhsT=consts["w0b"][:],
                         rhs=aggrT[:], start=False, stop=True)
        h1 = spool.tile([P, 128], f32, tag="h1", name="h1")
        nc.scalar.activation(h1[:], ph1[:], Act.Relu, bias=consts["b0"][:])

        ph2 = psmlp.tile([P, 128], f32, tag="mlp", name="mlp")
        nc.tensor.matmul(out=ph2[:], lhsT=consts["w1"][:], rhs=h1[:],
                         start=True, stop=True)
        h2 = spool.tile([P, 128], f32, tag="h2", name="h2")
        nc.scalar.activation(h2[:], ph2[:], Act.Relu, bias=consts["b1"][:])

        ph3 = psmlp.tile([P, 128], f32, tag="mlp", name="mlp")
        nc.tensor.matmul(out=ph3[:], lhsT=consts["w2"][:], rhs=h2[:],
                         start=True, stop=True)
        h3T = spool.tile([P, 128], f32, tag="h3T", name="h3T")
        nc.scalar.activation(h3T[:], ph3[:], Act.Identity,
                             bias=consts["b2"][:])

        # transpose to node-major, then LayerNorm over features
        py = psmlp.tile([P, 128], f32, tag="mlp", name="mlp")
        nc.tensor.transpose(py[:], h3T[:], ident[:])
        y = spool.tile([P, 128], f32, tag="y", name="y")
        nc.scalar.copy(y[:], py[:])

        stats = spool.tile([P, 6], f32, tag="stats", name="stats")
        nc.vector.bn_stats(stats[:], y[:])
        mv = spool.tile([P, 2], f32, tag="mv", name="mv")
        nc.vector.bn_aggr(mv[:], stats[:])
        std = spool.tile([P, 1], f32, tag="std", name="std")
        nc.scalar.activation(std[:], mv[:, 1:2], Act.Sqrt, bias=epst[:])
        rstd = spool.tile([P, 1], f32, tag="rstd", name="rstd")
        nc.vector.reciprocal(rstd[:], std[:])
        xn = spool.tile([P, 128], f32, tag="xn", name="xn")
        nc.vector.tensor_scalar(out=xn[:], in0=y[:], scalar1=mv[:, 0:1],
                                scalar2=rstd[:], op0=Alu.subtract,
                                op1=Alu.mult)
        g1 = spool.tile([P, 128], f32, tag="g1", name="g1")
        nc.gpsimd.tensor_tensor(out=g1[:], in0=xn[:], in1=consts["gam"][:],
                                op=Alu.mult)
        yo = spool.tile([P, 128], f32, tag="yo", name="yo")
        nc.gpsimd.tensor_tensor(out=yo[:], in0=g1[:], in1=consts["bet"][:],
                                op=Alu.add)
        odma.dma_start(out=out[b], in_=yo[:])
        c_off += K


def _prepare_shards(node_attr, edge_attr, col):
    """Bucket edges by destination-node block; build per-core arrays."""
    E = col.shape[0]
    blk = col >> 7                                  # global block id
    counts = np.bincount(blk, minlength=TOTAL_BLOCKS)
    kb = np.ceil(np.maximum(
        counts.reshape(N_CORES, BLOCKS_PER_CORE).max(axis=0), 1) / 128
    ).astype(np.int64)                              # [100] per-position tiles
    kbe = kb * 128
    blk_start = np.zeros(BLOCKS_PER_CORE + 1, np.int64)
    blk_start[1:] = np.cumsum(kbe)                  # slot base per position
    slots_per_core = int(blk_start[-1])

    order = np.argsort(blk, kind="stable")
    starts = np.zeros(TOTAL_BLOCKS + 1, np.int64)
    starts[1:] = np.cumsum(counts)
    blk_sorted = blk[order]
    within = np.arange(E, dtype=np.int64) - starts[blk_sorted]
    col_local_sorted = (col[order] & 127).astype(np.float16)

    # edges layout per core: [P, sum_b 2*kbe[b]] fp16 (hi block then lo block)
    edges_by_core = []
    colf_by_core = []
    natp = np.zeros((N_CORES * NODES_PER_CORE, D), np.float32)
    natp[:NUM_NODES] = node_attr
    natT_by_core = []
    for c in range(N_CORES):
        lo_i = int(starts[c * BLOCKS_PER_CORE])
        hi_i = int(starts[(c + 1) * BLOCKS_PER_CORE])
        loc_blk = blk_sorted[lo_i:hi_i] - c * BLOCKS_PER_CORE
        slot = blk_start[loc_blk] + within[lo_i:hi_i]
        ebuf = np.zeros((slots_per_core, D), np.float32)
        ebuf[slot] = edge_attr[order[lo_i:hi_i]]
        ehi = ebuf.astype(np.float16)
        elo = (ebuf - ehi.astype(np.float32)).astype(np.float16)
        cbuf = np.full((slots_per_core,), -1.0, np.float16)
        cbuf[slot] = col_local_sorted[lo_i:hi_i]

        earr = np.empty((P, int(2 * kbe.sum())), np.float16)
        carr = np.empty((P, int(kb.sum())), np.float16)
        e_off = 0
        c_off = 0
        for b in range(BLOCKS_PER_CORE):
            s0, s1 = int(blk_start[b]), int(blk_start[b + 1])
            K = int(kb[b])
            KE = K * 128
            earr[:, e_off:e_off + KE] = (
                ehi[s0:s1].reshape(K, 128, D).transpose(1, 0, 2)
                .reshape(P, K * 128))
            earr[:, e_off + KE:e_off + 2 * KE] = (
                elo[s0:s1].reshape(K, 128, D).transpose(1, 0, 2)
                .reshape(P, K * 128))
            carr[:, c_off:c_off + K] = cbuf[s0:s1].reshape(K, 128).T
            e_off += 2 * KE
            c_off += K
        edges_by_core.append(earr)
        colf_by_core.append(carr)
        natT_by_core.append(np.ascontiguousarray(
            natp[c * NODES_PER_CORE:(c + 1) * NODES_PER_CORE].T))
    return tuple(int(x) for x in kb), edges_by_core, colf_by_core, \
        natT_by_core


def kernel(node_attr, edge_attr, edge_index, W0, b0, W1, b1, W2, b2,
           ln_g, ln_b):
    from concourse import bass_utils

    node_attr = np.ascontiguousarray(np.asarray(node_attr, dtype=np.float32))
    edge_attr = np.ascontiguousarray(np.asarray(edge_attr, dtype=np.float32))
    col = np.asarray(edge_index)[1].astype(np.int64)
    W0 = np.asarray(W0, dtype=np.float32)
    W1 = np.ascontiguousarray(np.asarray(W1, dtype=np.float32))
    W2 = np.ascontiguousarray(np.asarray(W2, dtype=np.float32))
    b0v = np.asarray(b0, dtype=np.float32).reshape(128, 1).copy()
    b1v = np.asarray(b1, dtype=np.float32).reshape(128, 1).copy()
    b2v = np.asarray(b2, dtype=np.float32).reshape(128, 1).copy()
    gam = np.ascontiguousarray(
        np.tile(np.asarray(ln_g, np.float32).reshape(1, 128), (128, 1)))
    bet = np.ascontiguousarray(
        np.tile(np.asarray(ln_b, np.float32).reshape(1, 128), (128, 1)))

    kb, edges_by_core, colf_by_core, natT_by_core = _prepare_shards(
        node_attr, edge_attr, col)
    kmax = max(kb)

    iota_rep = np.ascontiguousarray(
        np.broadcast_to(np.arange(128, dtype=np.float16), (P, kmax, 128)))
    w0a = np.ascontiguousarray(W0[:128])
    w0b = np.ascontiguousarray(W0[128:])

    if kb not in _nc_cache:
        _nc_cache[kb] = _build_nc(kb)
    nc = _nc_cache[kb]

    shared = {"iota": iota_rep, "w0a": w0a, "w0b": w0b, "w1": W1, "w2": W2,
              "b0": b0v, "b1": b1v, "b2": b2v, "gam": gam, "bet": bet}
    in_maps = []
    for c in range(N_CORES):
        m = {"edges": edges_by_core[c], "colf": colf_by_core[c],
             "colf32": colf_by_core[c].astype(np.float32),
             "natT": natT_by_core[c]}
        m.update(shared)
        in_maps.append(m)

    res = bass_utils.run_bass_kernel_spmd(nc, in_maps,
                                          core_ids=list(range(N_CORES)))
    last_run_info["results"] = res
    last_run_info["nc"] = nc
    last_run_info["in_maps"] = in_maps
    last_run_info["kb"] = kb

    outs = [res.results[c]["out"].reshape(NODES_PER_CORE, D)
            for c in range(N_CORES)]
    return np.concatenate(outs, axis=0)[:NUM_NODES].astype(np.float32)



# revision 8
# speedup vs baseline: 1.1308x; 1.1308x over previous
"""Trainium2 Bass kernel for nn_NodeBlock (GNN message passing), v2.

Pipeline: segment_sum of edge features onto destination nodes, concat with
node features, 3-layer MLP, LayerNorm.

Sharding: nodes are assigned to 800 blocks of 128 by LPT bin-packing on
degree so every block has ~2048 incident edges; blocks are dealt to the 8
cores (100 each).  Edges are bucketed by destination block on the host and
stored fp16 (single precision stream; ~1e-4 rel err, gate is 2e-2), padded
per block to a uniform K tiles of 128 edge slots.

Per 128-node block the segment-sum is K one-hot fp16 matmuls accumulated in
PSUM (aggrT[f, j] = sum_e e[e, f] * (col[e] == j)).  Four blocks form a
superblock (512 nodes) processed by one fused fp16 MLP + LayerNorm epilogue:
mean is removed by PE rank-1 accumulation into the PSUM holding h3, gamma is
folded into the PE transpose (matmul with diag(gamma)), variance comes from
per-block stationary matmuls against a 1/128 ones column, and the output is
written back feature-interleaved fp16; the host undoes the node permutation.
"""

import sys

sys.path.insert(0, "/opt/trn_rl_repo")

import numpy as np

N_CORES = 8
NUM_NODES = 100000
D = 128            # node/edge feature dim
P = 128            # partitions
BLK = 128          # nodes per block
BLOCKS_PER_CORE = 100
SB = 4             # blocks per superblock
SBLOCKS = BLOCKS_PER_CORE // SB          # 25
NODES_PER_CORE = BLK * BLOCKS_PER_CORE   # 12800
TOTAL_BLOCKS = N_CORES * BLOCKS_PER_CORE  # 800
EPS = 1e-5

_nc_cache = {}
last_run_info = {}

TUNE = {"ebufs": 2, "ohbufs": 8, "sbufs": 3, "agbufs": 2, "mlpbufs": 2,
        "pybufs": 2, "dma_split": True, "beta_engine": "gp",
        "gp_oh": 0, "only": None}


def _build_nc(kb, loop_iters=None):
    """kb: tuple of per-block edge-tile counts (len 100, uniform in v2)."""
    import contextlib
    import concourse.bacc as bacc
    import concourse.tile as tile
    import concourse.mybir as mybir

    dt = mybir.dt
    f32 = dt.float32
    f16 = dt.float16
    kb = list(kb)
    tot_t = sum(kb)                   # total edge tiles per core
    tot_e = tot_t * 128               # per-partition fp16 elems

    nc = bacc.Bacc("TRN2", target_bir_lowering=False, debug=False,
                   name="nodeblock")

    edges = nc.dram_tensor("edges", [P, tot_e], f16, kind="ExternalInput")
    colf32 = nc.dram_tensor("colf32", [P, tot_t], f32, kind="ExternalInput")
    natT = nc.dram_tensor("natT", [P, NODES_PER_CORE], f16,
                          kind="ExternalInput")
    iota = nc.dram_tensor("iota", [P, 128], f16, kind="ExternalInput")
    w_in = {}
    for nm in ["w0a", "w0b", "w1", "w2", "gdiag"]:
        w_in[nm] = nc.dram_tensor(nm, [128, 128], f16, kind="ExternalInput")
    for nm in ["b0", "b1", "b2"]:
        w_in[nm] = nc.dram_tensor(nm, [128, 1], f32, kind="ExternalInput")
    w_in["b2row"] = nc.dram_tensor("b2row", [1, 128], f16,
                                   kind="ExternalInput")
    w_in["bet"] = nc.dram_tensor("bet", [128, SB, 128], f16,
                                 kind="ExternalInput")
    out = nc.dram_tensor("out", [SBLOCKS, P, SB, 128], f16,
                         kind="ExternalOutput")

    with tile.TileContext(nc) as tc:
        with (
            tc.tile_pool(name="const", bufs=1) as cpool,
            tc.tile_pool(name="edge", bufs=TUNE["ebufs"]) as epool,
            tc.tile_pool(name="oh", bufs=TUNE["ohbufs"]) as ohpool,
            tc.tile_pool(name="small", bufs=TUNE["sbufs"]) as spool,
            tc.tile_pool(name="psag", bufs=TUNE["agbufs"],
                         space="PSUM") as psag,
            tc.tile_pool(name="psmlp", bufs=TUNE["mlpbufs"],
                         space="PSUM") as psmlp,
            tc.tile_pool(name="pspy", bufs=TUNE["pybufs"],
                         space="PSUM") as pspy,
            tc.tile_pool(name="psst", bufs=1, space="PSUM") as psst,
        ):
            cdma = nc.scalar if TUNE["dma_split"] else nc.sync
            colf_s = cpool.tile([P, tot_t], f32, tag="colf32", name="colf32")
            cdma.dma_start(out=colf_s[:], in_=colf32[:])
            natT_s = cpool.tile([P, NODES_PER_CORE], f16, tag="natT",
                                name="natT")
            cdma.dma_start(out=natT_s[:], in_=natT[:])
            iota_s = cpool.tile([P, 128], f16, tag="iota", name="iota")
            cdma.dma_start(out=iota_s[:], in_=iota[:])
            consts = {}
            for nm, t in w_in.items():
                dtt = f16 if nm in ("w0a", "w0b", "w1", "w2", "gdiag",
                                    "b2row", "bet") else f32
                consts[nm] = cpool.tile(list(t.shape), dtt, tag=nm, name=nm)
                cdma.dma_start(out=consts[nm][:], in_=t[:])
            onesc = cpool.tile([P, 1], f16, tag="onesc", name="onesc")
            nc.vector.memset(onesc[:], 1.0 / 128.0)
            negones = cpool.tile([1, 128], f16, tag="negones", name="negones")
            nc.vector.memset(negones[:], -1.0)
            ones512 = cpool.tile([1, SB * 128], f16, tag="ones512",
                                 name="ones512")
            nc.vector.memset(ones512[:], 1.0)
            epst = cpool.tile([P, 1], f32, tag="eps", name="eps")
            nc.vector.memset(epst[:], EPS)

            loop_cm = (tc.For_i(0, loop_iters, 1) if loop_iters
                       else contextlib.nullcontext())
            with loop_cm:
                _emit(nc, tc, kb, epool, ohpool, spool, psag, psmlp, pspy,
                      psst, colf_s, natT_s, iota_s, consts, onesc, negones,
                      ones512, epst, edges, out, mybir)
    nc.finalize()
    return nc


def _emit(nc, tc, kb, epool, ohpool, spool, psag, psmlp, pspy, psst, colf_s,
          natT_s, iota_s, consts, onesc, negones, ones512, epst, edges, out,
          mybir):
    dt = mybir.dt
    f32 = dt.float32
    f16 = dt.float16
    Alu = mybir.AluOpType
    Act = mybir.ActivationFunctionType
    only = TUNE["only"]
    gp_oh = TUNE["gp_oh"]
    e_off = 0
    t_off = 0
    edma = nc.sync
    odma = nc.scalar if TUNE["dma_split"] else nc.sync

    eblk0 = None
    oh0 = None
    if only == "pe":
        kmax = max(kb)
        eblk0 = epool.tile([P, SB * kmax * 128], f16, tag="eblk", name="eblk")
        nc.sync.dma_start(out=eblk0[:], in_=edges[:, :SB * kmax * 128])
        oh0 = ohpool.tile([P, 128], f16, tag="oh", name="oh")
        nc.vector.tensor_scalar(out=oh0[:], in0=iota_s[:],
                                scalar1=colf_s[:, 0:1], scalar2=None,
                                op0=Alu.is_equal)

    for s in range(SBLOCKS):
        ks = kb[s * SB:(s + 1) * SB]
        sb_tiles = sum(ks)
        sb_e = sb_tiles * 128
        if only in (None, "dma", "agg"):
            eblk = epool.tile([P, sb_e], f16, tag="eblk", name="eblk")
            edma.dma_start(out=eblk[:], in_=edges[:, e_off:e_off + sb_e])
        else:
            eblk = eblk0
        e_off += sb_e
        if only == "dma":
            t_off += sb_tiles
            continue

        if only == "dve":
            for k in range(sb_tiles):
                oh = ohpool.tile([P, 128], f16, tag="oh", name="oh")
                nc.vector.tensor_scalar(
                    out=oh[:], in0=iota_s[:],
                    scalar1=colf_s[:, t_off + k:t_off + k + 1],
                    scalar2=None, op0=Alu.is_equal)
            t_off += sb_tiles
            continue

        # aggregation: one PSUM tile [P, 512] for the 4 blocks
        pag = psag.tile([P, SB * 128], f32, tag="ag", name="ag")
        kk = 0
        for b4 in range(SB):
            K = ks[b4]
            for k in range(K):
                if only == "pe":
                    oh = oh0
                else:
                    oh = ohpool.tile([P, 128], f16, tag="oh", name="oh")
                    if gp_oh and (k % (16 // max(gp_oh, 1))) == 0:
                        eng = nc.gpsimd
                    else:
                        eng = nc.vector
                    eng.tensor_scalar(
                        out=oh[:], in0=iota_s[:],
                        scalar1=colf_s[:, t_off + kk:t_off + kk + 1],
                        scalar2=None, op0=Alu.is_equal)
                nc.tensor.matmul(out=pag[:, b4 * 128:(b4 + 1) * 128],
                                 lhsT=eblk[:, kk * 128:(kk + 1) * 128],
                                 rhs=oh[:], start=(k == 0), stop=(k == K - 1))
                kk += 1
        t_off += sb_tiles
        if only in ("agg", "pe"):
            continue

        aggrT = spool.tile([P, SB * 128], f16, tag="aggrT", name="aggrT")
        nc.scalar.copy(aggrT[:], pag[:])

        # MLP (fp16 weights, fp32 PSUM accumulate)
        ph1 = psmlp.tile([P, SB * 128], f32, tag="mlp", name="mlp")
        nc.tensor.matmul(out=ph1[:], lhsT=consts["w0a"][:],
                         rhs=natT_s[:, s * SB * 128:(s + 1) * SB * 128],
                         start=True, stop=False)
        nc.tensor.matmul(out=ph1[:], lhsT=consts["w0b"][:], rhs=aggrT[:],
                         start=False, stop=True)
        h1 = spool.tile([P, SB * 128], f16, tag="h1", name="h1")
        nc.scalar.activation(h1[:], ph1[:], Act.Relu, bias=consts["b0"][:])

        ph2 = psmlp.tile([P, SB * 128], f32, tag="mlp", name="mlp")
        nc.tensor.matmul(out=ph2[:], lhsT=consts["w1"][:], rhs=h1[:],
                         start=True, stop=True)
        h2 = spool.tile([P, SB * 128], f16, tag="h2", name="h2")
        nc.scalar.activation(h2[:], ph2[:], Act.Relu, bias=consts["b1"][:])

        ph3 = psmlp.tile([P, SB * 128], f32, tag="mlp", name="mlp")
        nc.tensor.matmul(out=ph3[:], lhsT=consts["w2"][:], rhs=h2[:],
                         start=True, stop=False)
        # h3 (with bias) for the mean
        h3T = spool.tile([P, SB * 128], f16, tag="h3T", name="h3T")
        nc.scalar.activation(h3T[:], ph3[:], Act.Identity,
                             bias=consts["b2"][:])
        mur = psst.tile([1, SB * 128], f32, tag="mur", name="mur")
        nc.tensor.matmul(out=mur[:], lhsT=onesc[:], rhs=h3T[:],
                         start=True, stop=True)
        mu_sb = spool.tile([1, SB * 128], f16, tag="mu", name="mu")
        nc.scalar.copy(mu_sb[:], mur[:])
        # centered = ph3 + b2 - mu  (two rank-1 accumulations)
        nc.tensor.matmul(out=ph3[:], lhsT=consts["b2row"][:], rhs=ones512[:],
                         start=False, stop=False)
        nc.tensor.matmul(out=ph3[:], lhsT=negones[:], rhs=mu_sb[:],
                         start=False, stop=True)
        cT = spool.tile([P, SB * 128], f16, tag="cT", name="cT")
        nc.scalar.copy(cT[:], ph3[:])
        sq = spool.tile([P, SB * 128], f16, tag="sq", name="sq")
        nc.scalar.activation(sq[:], ph3[:], Act.Square)

        pvc = psst.tile([P, SB], f32, tag="vc", name="vc")
        for b4 in range(SB):
            nc.tensor.matmul(out=pvc[:, b4:b4 + 1],
                             lhsT=sq[:, b4 * 128:(b4 + 1) * 128],
                             rhs=onesc[:], start=True, stop=True)
        std = spool.tile([P, SB], f32, tag="std", name="std")
        nc.scalar.activation(std[:], pvc[:], Act.Sqrt, bias=epst[:])
        rstd = spool.tile([P, SB], f32, tag="rstd", name="rstd")
        nc.vector.reciprocal(rstd[:], std[:])

        pyt = pspy.tile([P, SB, 128], f32, tag="py", name="py")
        for b4 in range(SB):
            nc.tensor.matmul(out=pyt[:, b4, :],
                             lhsT=cT[:, b4 * 128:(b4 + 1) * 128],
                             rhs=consts["gdiag"][:], start=True, stop=True)
        xn = spool.tile([P, SB, 128], f16, tag="xn", name="xn")
        nc.vector.tensor_tensor(
            out=xn[:], in0=pyt[:],
            in1=rstd[:].broadcast_to([P, SB, 128]), op=Alu.mult)
        yo = spool.tile([P, SB, 128], f16, tag="yo", name="yo")
        beng = nc.gpsimd if TUNE["beta_engine"] == "gp" else nc.vector
        beng.tensor_tensor(out=yo[:], in0=xn[:], in1=consts["bet"][:],
                           op=Alu.add)
        odma.dma_start(out=out[s], in_=yo[:])


def _prepare_shards(node_attr, edge_attr, col):
    """LPT node->block packing + per-core edge bucketing (fp16)."""
    import heapq

    deg = np.bincount(col, minlength=NUM_NODES).astype(np.int64)
    order_nodes = np.argsort(-deg, kind="stable")
    # LPT: deal nodes (desc degree) into 800 blocks, 128 nodes each,
    # minimizing max block edge count.
    heap = [(0, 0, b) for b in range(TOTAL_BLOCKS)]
    heapq.heapify(heap)
    block_nodes = [[] for _ in range(TOTAL_BLOCKS)]
    for nd in order_nodes:
        d = int(deg[nd])
        s, cnt, b = heapq.heappop(heap)
        block_nodes[b].append(int(nd))
        if cnt + 1 < BLK:
            heapq.heappush(heap, (s + d, cnt + 1, b))
    sums = np.array([deg[bn].sum() for bn in block_nodes])
    K = int(np.ceil(max(1, sums.max()) / 128.0))
    kb = (K,) * BLOCKS_PER_CORE

    # node permutation: new id = block*128 + idx
    perm = np.full(TOTAL_BLOCKS * BLK, -1, dtype=np.int64)  # new -> old
    pos = np.full(NUM_NODES, -1, dtype=np.int64)            # old -> new
    for b, bn in enumerate(block_nodes):
        for i, nd in enumerate(bn):
            perm[b * BLK + i] = nd
            pos[nd] = b * BLK + i
    assert (pos >= 0).all()

    natp = np.zeros((TOTAL_BLOCKS * BLK, D), np.float16)
    valid = perm >= 0
    natp[valid] = node_attr[perm[valid]].astype(np.float16)

    npos = pos[col]
    blk = npos >> 7
    loc = (npos & 127).astype(np.float32)
    order = np.argsort(blk, kind="stable")
    counts = np.bincount(blk, minlength=TOTAL_BLOCKS)
    assert counts.max() <= K * 128
    starts = np.zeros(TOTAL_BLOCKS + 1, np.int64)
    starts[1:] = np.cumsum(counts)
    within = np.arange(col.shape[0], dtype=np.int64) - starts[blk[order]]

    slots_per_core = BLOCKS_PER_CORE * K * 128
    edges_by_core = []
    colf_by_core = []
    natT_by_core = []
    ea16 = edge_attr.astype(np.float16)
    for c in range(N_CORES):
        lo_i = int(starts[c * BLOCKS_PER_CORE])
        hi_i = int(starts[(c + 1) * BLOCKS_PER_CORE])
        sel = order[lo_i:hi_i]
        loc_blk = blk[sel] - c * BLOCKS_PER_CORE
        slot = loc_blk * (K * 128) + within[lo_i:hi_i]
        ebuf = np.zeros((slots_per_core, D), np.float16)
        ebuf[slot] = ea16[sel]
        cbuf = np.full((slots_per_core,), -1.0, np.float32)
        cbuf[slot] = loc[sel]
        earr = np.ascontiguousarray(
            ebuf.reshape(BLOCKS_PER_CORE * K, 128, D)
            .transpose(1, 0, 2).reshape(P, slots_per_core))
        carr = np.ascontiguousarray(
            cbuf.reshape(BLOCKS_PER_CORE * K, 128).T)
        edges_by_core.append(earr)
        colf_by_core.append(carr)
        natT_by_core.append(np.ascontiguousarray(
            natp[c * NODES_PER_CORE:(c + 1) * NODES_PER_CORE].T))
    return kb, edges_by_core, colf_by_core, natT_by_core, pos


def kernel(node_attr, edge_attr, edge_index, W0, b0, W1, b1, W2, b2,
           ln_g, ln_b):
    from concourse import bass_utils

    node_attr = np.ascontiguousarray(np.asarray(node_attr, dtype=np.float32))
    edge_attr = np.ascontiguousarray(np.asarray(edge_attr, dtype=np.float32))
    col = np.asarray(edge_index)[1].astype(np.int64)
    W0 = np.asarray(W0, dtype=np.float32)
    W1 = np.asarray(W1, dtype=np.float32)
    W2 = np.asarray(W2, dtype=np.float32)
    b0v = np.asarray(b0, np.float32).reshape(128, 1).copy()
    b1v = np.asarray(b1, np.float32).reshape(128, 1).copy()
    b2v = np.asarray(b2, np.float32).reshape(128, 1).copy()
    b2row = np.asarray(b2, np.float16).reshape(1, 128).copy()
    gdiag = np.ascontiguousarray(
        np.diag(np.asarray(ln_g, np.float32)).astype(np.float16))
    bet = np.ascontiguousarray(
        np.broadcast_to(np.asarray(ln_b, np.float32).reshape(1, 1, 128),
                        (128, SB, 128)).astype(np.float16))

    kb, edges_by_core, colf_by_core, natT_by_core, pos = _prepare_shards(
        node_attr, edge_attr, col)

    iota_rep = np.ascontiguousarray(
        np.broadcast_to(np.arange(128, dtype=np.float16), (P, 128)))
    w0a = np.ascontiguousarray(W0[:128].astype(np.float16))
    w0b = np.ascontiguousarray(W0[128:].astype(np.float16))

    if kb not in _nc_cache:
        _nc_cache[kb] = _build_nc(kb)
    nc = _nc_cache[kb]

    shared = {"iota": iota_rep, "w0a": w0a, "w0b": w0b,
              "w1": np.ascontiguousarray(W1.astype(np.float16)),
              "w2": np.ascontiguousarray(W2.astype(np.float16)),
              "gdiag": gdiag, "b0": b0v, "b1": b1v, "b2": b2v,
              "b2row": b2row, "bet": bet}
    in_maps = []
    for c in range(N_CORES):
        m = {"edges": edges_by_core[c], "colf32": colf_by_core[c],
             "natT": natT_by_core[c]}
        m.update(shared)
        in_maps.append(m)

    res = bass_utils.run_bass_kernel_spmd(nc, in_maps,
                                          core_ids=list(range(N_CORES)))
    last_run_info["results"] = res
    last_run_info["nc"] = nc
    last_run_info["in_maps"] = in_maps
    last_run_info["kb"] = kb
    last_run_info["pos"] = pos

    rows = np.concatenate(
        [res.results[c]["out"].reshape(SBLOCKS, P, SB, 128)
         .transpose(0, 2, 1, 3).reshape(NODES_PER_CORE, D)
         for c in range(N_CORES)], axis=0)
    return rows[pos].astype(np.float32)


if __name__ == "__main__":
    pass


# revision 10
# speedup vs baseline: 1.3092x; 1.1577x over previous
"""Trainium2 Bass kernel for nn_NodeBlock (GNN message passing), v3.

Pipeline: segment_sum of edge features onto destination nodes, concat with
node features, 3-layer MLP, LayerNorm.

Sharding: nodes are packed into 800 blocks of 128 by LPT on padded degree
(each node's edge list padded to a multiple of G=4), blocks dealt to 8
cores.  Edges are fp16 (single stream; ~6e-4 rel err vs the 2e-2 gate).

Segment sum is two-stage to cut DVE one-hot work 4x: per block, 16 "main"
edge tiles are pre-reduced on the PE with a fixed G=4 grouping matrix
(col-tiled 4x: 32-col matmuls at tile_position (0,32i) -> one [128,512]
PSUM of per-group sums), then 4 one-hot matmuls scatter the 512 groups onto
the 128 nodes.  Up to 2 remainder tiles per block (nodes that didn't fit
G-aligned) are scattered directly per-edge.  One-hots come from DVE
tensor_scalar is_equal against an iota row.

Four blocks form a superblock (512 nodes) processed by one fused fp16
MLP + LayerNorm epilogue: column stats (mean, E[x^2] per node) via
stationary matmuls against a 1/128 ones column, PE transposes to
node-major, normalize via DVE tensor_scalar, gamma/beta on DVE/GPSIMD.
Output is fp16, node-permuted; the host undoes the permutation.
"""

import sys

sys.path.insert(0, "/opt/trn_rl_repo")

import numpy as np

N_CORES = 8
NUM_NODES = 100000
D = 128            # node/edge feature dim
P = 128            # partitions
BLK = 128          # nodes per block
G = 4              # edges per pre-reduction group
KMAIN = 16         # main edge tiles per block (G-aligned region, 4 quads)
KREM = 2           # remainder edge tiles per block (direct one-hot)
KTOT = KMAIN + KREM
BLOCKS_PER_CORE = 100
SB = 4             # blocks per superblock
SBLOCKS = BLOCKS_PER_CORE // SB          # 25
NODES_PER_CORE = BLK * BLOCKS_PER_CORE   # 12800
TOTAL_BLOCKS = N_CORES * BLOCKS_PER_CORE  # 800
COLS_PER_BLOCK = SB + KREM               # 4 group cols + 2 rem cols
EPS = 1e-5

_nc_cache = {}
last_run_info = {}

TUNE = {"ebufs": 2, "ohbufs": 8, "sbufs": 3, "agbufs": 2, "mlpbufs": 1,
        "pqbufs": 2, "gam_engine": "gp", "beta_engine": "gp",
        "only": None}


def _build_nc(kb, loop_iters=None):
    """kb is kept for test.py compatibility; v3 uses fixed KTOT tiles."""
    import contextlib
    import concourse.bacc as bacc
    import concourse.tile as tile
    import concourse.mybir as mybir

    dt = mybir.dt
    f32 = dt.float32
    f16 = dt.float16
    tot_e = BLOCKS_PER_CORE * KTOT * 128

    nc = bacc.Bacc("TRN2", target_bir_lowering=False, debug=False,
                   name="nodeblock")

    edges = nc.dram_tensor("edges", [P, tot_e], f16, kind="ExternalInput")
    colf32 = nc.dram_tensor("colf32", [P, BLOCKS_PER_CORE * COLS_PER_BLOCK],
                            f32, kind="ExternalInput")
    natT = nc.dram_tensor("natT", [P, NODES_PER_CORE], f16,
                          kind="ExternalInput")
    iota = nc.dram_tensor("iota", [P, 128], f16, kind="ExternalInput")
    w_in = {}
    for nm in ["w0a", "w0b", "w1", "w2", "ident"]:
        w_in[nm] = nc.dram_tensor(nm, [128, 128], f16, kind="ExternalInput")
    for nm in ["b0", "b1", "b2"]:
        w_in[nm] = nc.dram_tensor(nm, [128, 1], f32, kind="ExternalInput")
    w_in["rmat"] = nc.dram_tensor("rmat", [128, 32], f16,
                                  kind="ExternalInput")
    w_in["gam"] = nc.dram_tensor("gam", [128, SB, 128], f16,
                                 kind="ExternalInput")
    w_in["bet"] = nc.dram_tensor("bet", [128, SB, 128], f16,
                                 kind="ExternalInput")
    out = nc.dram_tensor("out", [SBLOCKS, P, SB, 128], f16,
                         kind="ExternalOutput")

    with tile.TileContext(nc) as tc:
        with (
            tc.tile_pool(name="const", bufs=1) as cpool,
            tc.tile_pool(name="edge", bufs=TUNE["ebufs"]) as epool,
            tc.tile_pool(name="oh", bufs=TUNE["ohbufs"]) as ohpool,
            tc.tile_pool(name="small", bufs=TUNE["sbufs"]) as spool,
            tc.tile_pool(name="psag", bufs=TUNE["agbufs"],
                         space="PSUM") as psag,
            tc.tile_pool(name="psmlp", bufs=TUNE["mlpbufs"],
                         space="PSUM") as psmlp,
            tc.tile_pool(name="pspq", bufs=TUNE["pqbufs"],
                         space="PSUM") as pspq,
            tc.tile_pool(name="psaux", bufs=1, space="PSUM") as psaux,
        ):
            cdma = nc.scalar
            colf_s = cpool.tile([P, BLOCKS_PER_CORE * COLS_PER_BLOCK], f32,
                                tag="colf32", name="colf32")
            cdma.dma_start(out=colf_s[:], in_=colf32[:])
            natT_s = cpool.tile([P, NODES_PER_CORE], f16, tag="natT",
                                name="natT")
            cdma.dma_start(out=natT_s[:], in_=natT[:])
            iota_s = cpool.tile([P, 128], f16, tag="iota", name="iota")
            cdma.dma_start(out=iota_s[:], in_=iota[:])
            consts = {}
            for nm, t in w_in.items():
                dtt = f32 if nm in ("b0", "b1", "b2") else f16
                consts[nm] = cpool.tile(list(t.shape), dtt, tag=nm, name=nm)
                cdma.dma_start(out=consts[nm][:], in_=t[:])
            onesc = cpool.tile([P, 1], f16, tag="onesc", name="onesc")
            nc.vector.memset(onesc[:], 1.0 / 128.0)
            epst = cpool.tile([P, 1], f32, tag="eps", name="eps")
            nc.vector.memset(epst[:], EPS)

            loop_cm = (tc.For_i(0, loop_iters, 1) if loop_iters
                       else contextlib.nullcontext())
            with loop_cm:
                _emit(nc, tc, epool, ohpool, spool, psag, psmlp, pspq,
                      psaux, colf_s, natT_s, iota_s, consts, onesc, epst,
                      edges, out, mybir)
    nc.finalize()
    return nc


def _emit(nc, tc, epool, ohpool, spool, psag, psmlp, pspq, psaux, colf_s,
          natT_s, iota_s, consts, onesc, epst, edges, out, mybir):
    dt = mybir.dt
    f32 = dt.float32
    f16 = dt.float16
    Alu = mybir.AluOpType
    Act = mybir.ActivationFunctionType
    only = TUNE["only"]
    edma = nc.sync
    odma = nc.scalar
    R = consts["rmat"]
    sb_e = SB * KTOT * 128          # edge elems per superblock per partition

    for s in range(SBLOCKS):
        if only in (None, "dma", "agg", "s1"):
            eblk = epool.tile([P, sb_e], f16, tag="eblk", name="eblk")
            edma.dma_start(out=eblk[:], in_=edges[:, s * sb_e:(s + 1) * sb_e])
        if only == "dma":
            continue

        if only == "dve":
            for c in range(SB * COLS_PER_BLOCK):
                oh = ohpool.tile([P, 128], f16, tag="oh", name="oh")
                nc.vector.tensor_scalar(
                    out=oh[:], in0=iota_s[:],
                    scalar1=colf_s[:, s * SB * COLS_PER_BLOCK + c:
                                   s * SB * COLS_PER_BLOCK + c + 1],
                    scalar2=None, op0=Alu.is_equal)
            continue

        pag = psag.tile([P, SB * 128], f32, tag="ag", name="ag",
                        bufs=TUNE["agbufs"])
        for b4 in range(SB):
            ebase = (b4 * KTOT) * 128
            # stage 1: 16 main tiles -> [128, 512] groups (col-tiled 4x)
            pq = pspq.tile([P, 512], f32, tag="pq", name="pq",
                           bufs=TUNE["pqbufs"])
            for t in range(KMAIN):
                i, q = t % 4, t // 4
                nc.tensor.matmul(
                    out=pq[32 * i:32 * i + 32, 128 * q:128 * q + 128],
                    lhsT=R[:],
                    rhs=eblk[:, ebase + t * 128:ebase + (t + 1) * 128],
                    tile_position=(0, 32 * i), start=True, stop=True)
            if only == "s1":
                continue
            grp = spool.tile([P, 512], f16, tag="grp", name="grp")
            nc.scalar.copy(grp[:], pq[:])
            # stage 2: 4 group one-hot matmuls + 2 remainder edge tiles
            cbase = s * SB * COLS_PER_BLOCK + b4 * COLS_PER_BLOCK
            for q in range(SB):
                oh = ohpool.tile([P, 128], f16, tag="oh", name="oh")
                nc.vector.tensor_scalar(
                    out=oh[:], in0=iota_s[:],
                    scalar1=colf_s[:, cbase + q:cbase + q + 1],
                    scalar2=None, op0=Alu.is_equal)
                nc.tensor.matmul(out=pag[:, b4 * 128:(b4 + 1) * 128],
                                 lhsT=grp[:, q * 128:(q + 1) * 128],
                                 rhs=oh[:], start=(q == 0), stop=False)
            for r in range(KREM):
                oh = ohpool.tile([P, 128], f16, tag="oh", name="oh")
                nc.vector.tensor_scalar(
                    out=oh[:], in0=iota_s[:],
                    scalar1=colf_s[:, cbase + SB + r:cbase + SB + r + 1],
                    scalar2=None, op0=Alu.is_equal)
                nc.tensor.matmul(
                    out=pag[:, b4 * 128:(b4 + 1) * 128],
                    lhsT=eblk[:, ebase + (KMAIN + r) * 128:
                              ebase + (KMAIN + r + 1) * 128],
                    rhs=oh[:], start=False, stop=(r == KREM - 1))
        if only in ("agg", "s1"):
            continue

        aggrT = spool.tile([P, SB * 128], f16, tag="aggrT", name="aggrT")
        nc.scalar.copy(aggrT[:], pag[:])

        # MLP (fp16 weights, fp32 PSUM accumulate)
        ph1 = psmlp.tile([P, SB * 128], f32, tag="mlp", name="mlp")
        nc.tensor.matmul(out=ph1[:], lhsT=consts["w0a"][:],
                         rhs=natT_s[:, s * SB * 128:(s + 1) * SB * 128],
                         start=True, stop=False)
        nc.tensor.matmul(out=ph1[:], lhsT=consts["w0b"][:], rhs=aggrT[:],
                         start=False, stop=True)
        h1 = spool.tile([P, SB * 128], f16, tag="h1", name="h1")
        nc.scalar.activation(h1[:], ph1[:], Act.Relu, bias=consts["b0"][:])

        ph2 = psmlp.tile([P, SB * 128], f32, tag="mlp", name="mlp")
        nc.tensor.matmul(out=ph2[:], lhsT=consts["w1"][:], rhs=h1[:],
                         start=True, stop=True)
        h2 = spool.tile([P, SB * 128], f16, tag="h2", name="h2")
        nc.scalar.activation(h2[:], ph2[:], Act.Relu, bias=consts["b1"][:])

        ph3 = psmlp.tile([P, SB * 128], f32, tag="mlp", name="mlp")
        nc.tensor.matmul(out=ph3[:], lhsT=consts["w2"][:], rhs=h2[:],
                         start=True, stop=True)
        h3T = spool.tile([P, SB * 128], f16, tag="h3T", name="h3T")
        nc.scalar.activation(h3T[:], ph3[:], Act.Identity,
                             bias=consts["b2"][:])
        sq = spool.tile([P, SB * 128], f16, tag="sq", name="sq")
        nc.scalar.activation(sq[:], h3T[:], Act.Square)

        # column stats: mu and E[x^2] per node into one aux PSUM bank
        paux = psaux.tile([P, 2 * SB], f32, tag="aux", name="aux")
        for b4 in range(SB):
            nc.tensor.matmul(out=paux[:, b4:b4 + 1],
                             lhsT=h3T[:, b4 * 128:(b4 + 1) * 128],
                             rhs=onesc[:], start=True, stop=True)
        for b4 in range(SB):
            nc.tensor.matmul(out=paux[:, SB + b4:SB + b4 + 1],
                             lhsT=sq[:, b4 * 128:(b4 + 1) * 128],
                             rhs=onesc[:], start=True, stop=True)
        mu_sb = spool.tile([P, SB], f32, tag="mu", name="mu")
        nc.scalar.copy(mu_sb[:], paux[:, 0:SB])
        musq = spool.tile([P, SB], f32, tag="musq", name="musq")
        nc.scalar.activation(musq[:], paux[:, 0:SB], Act.Square)
        var = spool.tile([P, SB], f32, tag="var", name="var")
        nc.vector.tensor_tensor(out=var[:], in0=paux[:, SB:2 * SB],
                                in1=musq[:], op=Alu.subtract)
        std = spool.tile([P, SB], f32, tag="std", name="std")
        nc.scalar.activation(std[:], var[:], Act.Sqrt, bias=epst[:])
        rstd = spool.tile([P, SB], f32, tag="rstd", name="rstd")
        nc.vector.reciprocal(rstd[:], std[:])

        pyt = psag.tile([P, SB, 128], f32, tag="py", name="py",
                       bufs=2)
        for b4 in range(SB):
            nc.tensor.matmul(out=pyt[:, b4, :],
                             lhsT=h3T[:, b4 * 128:(b4 + 1) * 128],
                             rhs=consts["ident"][:], start=True, stop=True)
        xn = spool.tile([P, SB, 128], f16, tag="xn", name="xn")
        for b4 in range(SB):
            nc.vector.tensor_scalar(
                out=xn[:, b4, :], in0=pyt[:, b4, :],
                scalar1=mu_sb[:, b4:b4 + 1], scalar2=rstd[:, b4:b4 + 1],
                op0=Alu.subtract, op1=Alu.mult)
        geng = nc.gpsimd if TUNE["gam_engine"] == "gp" else nc.vector
        beng = nc.gpsimd if TUNE["beta_engine"] == "gp" else nc.vector
        yg = spool.tile([P, SB, 128], f16, tag="yg", name="yg")
        geng.tensor_tensor(out=yg[:], in0=xn[:], in1=consts["gam"][:],
                           op=Alu.mult)
        yo = spool.tile([P, SB, 128], f16, tag="yo", name="yo")
        beng.tensor_tensor(out=yo[:], in0=yg[:], in1=consts["bet"][:],
                           op=Alu.add)
        odma.dma_start(out=out[s], in_=yo[:])


def _prepare_shards(node_attr, edge_attr, col):
    """LPT node->block packing on G-padded degree + two-region edge layout."""
    import heapq

    deg = np.bincount(col, minlength=NUM_NODES).astype(np.int64)
    pdeg = ((deg + G - 1) // G) * G
    order_nodes = np.argsort(-pdeg, kind="stable")
    heap = [(0, 0, b) for b in range(TOTAL_BLOCKS)]
    heapq.heapify(heap)
    block_nodes = [[] for _ in range(TOTAL_BLOCKS)]
    for nd in order_nodes:
        d = int(pdeg[nd])
        s, cnt, b = heapq.heappop(heap)
        block_nodes[b].append(int(nd))
        if cnt + 1 < BLK:
            heapq.heappush(heap, (s + d, cnt + 1, b))

    main_cap = KMAIN * 128          # 2048 G-aligned slots
    rem_cap = KREM * 128            # 256 raw slots
    ngroups = main_cap // G         # 512

    # per-node placement: (block, local id, main slot base or -1, rem base)
    pos = np.full(NUM_NODES, -1, dtype=np.int64)      # old -> new node id
    edge_base = np.zeros(NUM_NODES, dtype=np.int64)   # slot of first edge
    natp = np.zeros((TOTAL_BLOCKS * BLK, D), np.float16)
    colg = np.full((TOTAL_BLOCKS, COLS_PER_BLOCK * 128), -1.0, np.float32)
    for b, bn in enumerate(block_nodes):
        # main region greedy: G-padded spans until 2048, rest raw
        mcur = 0
        rcur = 0
        for i, nd in enumerate(bn):
            pos[nd] = b * BLK + i
            natp[b * BLK + i] = node_attr[nd].astype(np.float16)
            d = int(deg[nd])
            pd = int(pdeg[nd])
            if mcur + pd <= main_cap:
                edge_base[nd] = b * KTOT * 128 + mcur
                g0, g1 = mcur // G, (mcur + pd) // G
                colg[b, g0:g1] = float(i)
                mcur += pd
            else:
                assert rcur + d <= rem_cap, (b, rcur, d)
                edge_base[nd] = b * KTOT * 128 + main_cap + rcur
                rcur += d
    assert (pos >= 0).all()

    # colf32 column layout per block: 4 group-quads then 2 remainder tiles.
    # group at partition p of quad q = (tile 4q + p//32, group p%32)
    #   -> flat group index g = (4q + p//32)*32 + p%32
    colf = np.full((TOTAL_BLOCKS, COLS_PER_BLOCK, 128), -1.0, np.float32)
    for q in range(SB):
        pidx = np.arange(128)
        g = (4 * q + pidx // 32) * 32 + pidx % 32
        colf[:, q, :] = colg[:, :ngroups][:, g]
    # remainder cols filled below from per-edge local node ids

    # place edges
    order = np.argsort(col, kind="stable")           # group edges per node
    cs = col[order]
    within = np.arange(col.shape[0], dtype=np.int64)
    starts = np.zeros(NUM_NODES + 1, np.int64)
    starts[1:] = np.cumsum(deg)
    within = within - starts[cs]
    slot = edge_base[cs] + within                    # global slot id

    ea16 = edge_attr.astype(np.float16)
    slots_per_core = BLOCKS_PER_CORE * KTOT * 128
    edges_by_core = []
    colf_by_core = []
    natT_by_core = []
    loc_of = (pos[cs] & 127).astype(np.float32)
    blk_of = slot // (KTOT * 128)
    off_of = slot % (KTOT * 128)
    for c in range(N_CORES):
        sel = (blk_of >= c * BLOCKS_PER_CORE) & \
              (blk_of < (c + 1) * BLOCKS_PER_CORE)
        lblk = blk_of[sel] - c * BLOCKS_PER_CORE
        lslot = lblk * (KTOT * 128) + off_of[sel]
        ebuf = np.zeros((slots_per_core, D), np.float16)
        ebuf[lslot] = ea16[order[sel]]
        earr = np.ascontiguousarray(
            ebuf.reshape(BLOCKS_PER_CORE * KTOT, 128, D)
            .transpose(1, 0, 2).reshape(P, slots_per_core))
        edges_by_core.append(earr)
        # remainder one-hot cols
        cf = colf[c * BLOCKS_PER_CORE:(c + 1) * BLOCKS_PER_CORE].copy()
        rm = off_of[sel] >= KMAIN * 128
        roff = off_of[sel][rm] - KMAIN * 128
        cf[lblk[rm], SB + roff // 128, roff % 128] = loc_of[sel][rm]
        carr = np.ascontiguousarray(
            cf.reshape(BLOCKS_PER_CORE * COLS_PER_BLOCK, 128).T)
        colf_by_core.append(carr)
        natT_by_core.append(np.ascontiguousarray(
            natp[c * NODES_PER_CORE:(c + 1) * NODES_PER_CORE].T))
    kb = (KTOT,) * BLOCKS_PER_CORE
    return kb, edges_by_core, colf_by_core, natT_by_core, pos


def kernel(node_attr, edge_attr, edge_index, W0, b0, W1, b1, W2, b2,
           ln_g, ln_b):
    from concourse import bass_utils

    node_attr = np.ascontiguousarray(np.asarray(node_attr, dtype=np.float32))
    edge_attr = np.ascontiguousarray(np.asarray(edge_attr, dtype=np.float32))
    col = np.asarray(edge_index)[1].astype(np.int64)
    W0 = np.asarray(W0, dtype=np.float32)
    W1 = np.asarray(W1, dtype=np.float32)
    W2 = np.asarray(W2, dtype=np.float32)
    b0v = np.asarray(b0, np.float32).reshape(128, 1).copy()
    b1v = np.asarray(b1, np.float32).reshape(128, 1).copy()
    b2v = np.asarray(b2, np.float32).reshape(128, 1).copy()
    gam = np.ascontiguousarray(
        np.broadcast_to(np.asarray(ln_g, np.float32).reshape(1, 1, 128),
                        (128, SB, 128)).astype(np.float16))
    bet = np.ascontiguousarray(
        np.broadcast_to(np.asarray(ln_b, np.float32).reshape(1, 1, 128),
                        (128, SB, 128)).astype(np.float16))

    kb, edges_by_core, colf_by_core, natT_by_core, pos = _prepare_shards(
        node_attr, edge_attr, col)

    iota_rep = np.ascontiguousarray(
        np.broadcast_to(np.arange(128, dtype=np.float16), (P, 128)))
    rmat = np.zeros((128, 32), np.float16)
    rmat[np.arange(128), np.arange(128) // G] = 1.0
    ident = np.eye(128, dtype=np.float16)

    if kb not in _nc_cache:
        _nc_cache[kb] = _build_nc(kb)
    nc = _nc_cache[kb]

    shared = {"iota": iota_rep, "rmat": rmat, "ident": ident,
              "w0a": np.ascontiguousarray(W0[:128].astype(np.float16)),
              "w0b": np.ascontiguousarray(W0[128:].astype(np.float16)),
              "w1": np.ascontiguousarray(W1.astype(np.float16)),
              "w2": np.ascontiguousarray(W2.astype(np.float16)),
              "gam": gam, "bet": bet, "b0": b0v, "b1": b1v, "b2": b2v}
    in_maps = []
    for c in range(N_CORES):
        m = {"edges": edges_by_core[c], "colf32": colf_by_core[c],
             "natT": natT_by_core[c]}
        m.update(shared)
        in_maps.append(m)

    res = bass_utils.run_bass_kernel_spmd(nc, in_maps,
                                          core_ids=list(range(N_CORES)))
    last_run_info["results"] = res
    last_run_info["nc"] = nc
    last_run_info["in_maps"] = in_maps
    last_run_info["kb"] = kb
    last_run_info["pos"] = pos

    rows = np.concatenate(
        [res.results[c]["out"].reshape(SBLOCKS, P, SB, 128)
         .transpose(0, 2, 1, 3).reshape(NODES_PER_CORE, D)
         for c in range(N_CORES)], axis=0)
    return rows[pos].astype(np.float32)


if __name__ == "__main__":
    pass


# revision 11
# speedup vs baseline: 1.5918x; 1.2159x over previous
"""Trainium2 Bass kernel for nn_NodeBlock (GNN message passing), v4.

Pipeline: segment_sum of edge features onto destination nodes, concat with
node features, 3-layer MLP, LayerNorm.

Layout: nodes are packed into 800 blocks of 128 (LPT on overflow degree),
blocks dealt to 8 cores.  Every node owns a FIXED span of 16 "main" edge
slots (4 groups of G=4); edges beyond 16 per node go to a per-block
remainder region (up to 256 slots).  Edges are fp16 (~7e-4 rel err vs the
2e-2 gate).

Segment sum is two-stage with constant matrices: per block, 16 main edge
tiles are pre-reduced on the PE with a fixed G=4 grouping matrix R
(col-tiled 4x: 32-col matmuls at tile_position (0,32i) -> one [128,512]
PSUM of per-group sums); because group->node is fixed, the scatter of the
512 groups onto 128 nodes is 4 matmuls against a CONSTANT selection matrix
S_q (no per-tile DVE work).  Only the <=256 remainder edges per block use
DVE one-hot (is_equal vs iota) scatter.

Four blocks form a superblock (512 nodes) with a fused fp16 MLP + LayerNorm
epilogue: column stats via stationary matmuls against a 1/128 ones column,
PE transposes to node-major, normalize via DVE tensor_scalar, gamma/beta
TTs.  Output is fp16, node-permuted; the host undoes the permutation.
"""

import sys

sys.path.insert(0, "/opt/trn_rl_repo")

import numpy as np

N_CORES = 8
NUM_NODES = 100000
D = 128            # node/edge feature dim
P = 128            # partitions
BLK = 128          # nodes per block
G = 4              # edges per pre-reduction group
CAP = 16           # main edge slots per node (4 groups)
KMAIN = 16         # main edge tiles per block (= BLK*CAP/128, 4 quads)
KREM = 2           # remainder edge tiles per block (direct one-hot)
KTOT = KMAIN + KREM
BLOCKS_PER_CORE = 100
SB = 4             # blocks per superblock
SBLOCKS = BLOCKS_PER_CORE // SB          # 25
NODES_PER_CORE = BLK * BLOCKS_PER_CORE   # 12800
TOTAL_BLOCKS = N_CORES * BLOCKS_PER_CORE  # 800
EPS = 1e-5

_nc_cache = {}
last_run_info = {}

TUNE = {"ebufs": 2, "ohbufs": 8, "sbufs": 3, "agbufs": 2, "mlpbufs": 1,
        "pqbufs": 2, "gam_engine": "dve", "beta_engine": "dve",
        "sq_engine": "dve", "grp_dve": 2, "only": None}


def _build_nc(kb, loop_iters=None):
    """kb is kept for test.py compatibility; v4 uses fixed KTOT tiles."""
    import contextlib
    import concourse.bacc as bacc
    import concourse.tile as tile
    import concourse.mybir as mybir

    dt = mybir.dt
    f32 = dt.float32
    f16 = dt.float16
    tot_e = BLOCKS_PER_CORE * KTOT * 128

    nc = bacc.Bacc("TRN2", target_bir_lowering=False, debug=False,
                   name="nodeblock")

    edges = nc.dram_tensor("edges", [P, tot_e], f16, kind="ExternalInput")
    colf32 = nc.dram_tensor("colf32", [P, BLOCKS_PER_CORE * KREM],
                            f32, kind="ExternalInput")
    natT = nc.dram_tensor("natT", [P, NODES_PER_CORE], f16,
                          kind="ExternalInput")
    iota = nc.dram_tensor("iota", [P, 128], f16, kind="ExternalInput")
    w_in = {}
    for nm in ["w0a", "w0b", "w1", "w2", "ident"]:
        w_in[nm] = nc.dram_tensor(nm, [128, 128], f16, kind="ExternalInput")
    for nm in ["b0", "b1", "b2"]:
        w_in[nm] = nc.dram_tensor(nm, [128, 1], f32, kind="ExternalInput")
    w_in["rmat"] = nc.dram_tensor("rmat", [128, 32], f16,
                                  kind="ExternalInput")
    w_in["smat"] = nc.dram_tensor("smat", [128, SB, 128], f16,
                                  kind="ExternalInput")
    w_in["gam"] = nc.dram_tensor("gam", [128, SB, 128], f16,
                                 kind="ExternalInput")
    w_in["bet"] = nc.dram_tensor("bet", [128, SB, 128], f16,
                                 kind="ExternalInput")
    out = nc.dram_tensor("out", [SBLOCKS, P, SB, 128], f16,
                         kind="ExternalOutput")

    with tile.TileContext(nc) as tc:
        with (
            tc.tile_pool(name="const", bufs=1) as cpool,
            tc.tile_pool(name="edge", bufs=TUNE["ebufs"]) as epool,
            tc.tile_pool(name="oh", bufs=TUNE["ohbufs"]) as ohpool,
            tc.tile_pool(name="small", bufs=TUNE["sbufs"]) as spool,
            tc.tile_pool(name="psag", bufs=TUNE["agbufs"],
                         space="PSUM") as psag,
            tc.tile_pool(name="psmlp", bufs=TUNE["mlpbufs"],
                         space="PSUM") as psmlp,
            tc.tile_pool(name="pspq", bufs=TUNE["pqbufs"],
                         space="PSUM") as pspq,
            tc.tile_pool(name="psaux", bufs=1, space="PSUM") as psaux,
        ):
            cdma = nc.scalar
            colf_s = cpool.tile([P, BLOCKS_PER_CORE * KREM], f32,
                                tag="colf32", name="colf32")
            cdma.dma_start(out=colf_s[:], in_=colf32[:])
            natT_s = cpool.tile([P, NODES_PER_CORE], f16, tag="natT",
                                name="natT")
            cdma.dma_start(out=natT_s[:], in_=natT[:])
            iota_s = cpool.tile([P, 128], f16, tag="iota", name="iota")
            cdma.dma_start(out=iota_s[:], in_=iota[:])
            consts = {}
            for nm, t in w_in.items():
                dtt = f32 if nm in ("b0", "b1", "b2") else f16
                consts[nm] = cpool.tile(list(t.shape), dtt, tag=nm, name=nm)
                cdma.dma_start(out=consts[nm][:], in_=t[:])
            onesc = cpool.tile([P, 1], f16, tag="onesc", name="onesc")
            nc.vector.memset(onesc[:], 1.0 / 128.0)
            epst = cpool.tile([P, 1], f32, tag="eps", name="eps")
            nc.vector.memset(epst[:], EPS)

            loop_cm = (tc.For_i(0, loop_iters, 1) if loop_iters
                       else contextlib.nullcontext())
            with loop_cm:
                _emit(nc, tc, epool, ohpool, spool, psag, psmlp, pspq,
                      psaux, colf_s, natT_s, iota_s, consts, onesc, epst,
                      edges, out, mybir)
    nc.finalize()
    return nc


def _emit(nc, tc, epool, ohpool, spool, psag, psmlp, pspq, psaux, colf_s,
          natT_s, iota_s, consts, onesc, epst, edges, out, mybir):
    dt = mybir.dt
    f32 = dt.float32
    f16 = dt.float16
    Alu = mybir.AluOpType
    Act = mybir.ActivationFunctionType
    only = TUNE["only"]
    edma = nc.sync
    odma = nc.scalar
    R = consts["rmat"]
    S = consts["smat"]
    sb_e = SB * KTOT * 128          # edge elems per superblock per partition

    for s in range(SBLOCKS):
        if only in (None, "dma", "agg", "s1"):
            eblk = epool.tile([P, sb_e], f16, tag="eblk", name="eblk")
            edma.dma_start(out=eblk[:], in_=edges[:, s * sb_e:(s + 1) * sb_e])
        if only == "dma":
            continue

        if only == "dve":
            for c in range(SB * KREM):
                oh = ohpool.tile([P, 128], f16, tag="oh", name="oh")
                nc.vector.tensor_scalar(
                    out=oh[:], in0=iota_s[:],
                    scalar1=colf_s[:, s * SB * KREM + c:
                                   s * SB * KREM + c + 1],
                    scalar2=None, op0=Alu.is_equal)
            continue

        pag = psag.tile([P, SB * 128], f32, tag="ag", name="ag",
                        bufs=TUNE["agbufs"])
        for b4 in range(SB):
            ebase = (b4 * KTOT) * 128
            # stage 1: 16 main tiles -> [128, 512] groups (col-tiled 4x)
            pq = pspq.tile([P, 512], f32, tag="pq", name="pq",
                           bufs=TUNE["pqbufs"])
            for t in range(KMAIN):
                i, q = t % 4, t // 4
                nc.tensor.matmul(
                    out=pq[32 * i:32 * i + 32, 128 * q:128 * q + 128],
                    lhsT=R[:],
                    rhs=eblk[:, ebase + t * 128:ebase + (t + 1) * 128],
                    tile_position=(0, 32 * i), start=True, stop=True)
            if only == "s1":
                continue
            grp = spool.tile([P, 512], f16, tag="grp", name="grp")
            geng = nc.vector if b4 < TUNE["grp_dve"] else nc.scalar
            if geng is nc.vector:
                nc.vector.tensor_copy(grp[:], pq[:])
            else:
                nc.scalar.copy(grp[:], pq[:])
            # stage 2: 4 constant-scatter matmuls + 2 remainder edge tiles
            cbase = s * SB * KREM + b4 * KREM
            for q in range(SB):
                nc.tensor.matmul(out=pag[:, b4 * 128:(b4 + 1) * 128],
                                 lhsT=grp[:, q * 128:(q + 1) * 128],
                                 rhs=S[:, q, :], start=(q == 0), stop=False)
            for r in range(KREM):
                oh = ohpool.tile([P, 128], f16, tag="oh", name="oh")
                nc.vector.tensor_scalar(
                    out=oh[:], in0=iota_s[:],
                    scalar1=colf_s[:, cbase + r:cbase + r + 1],
                    scalar2=None, op0=Alu.is_equal)
                nc.tensor.matmul(
                    out=pag[:, b4 * 128:(b4 + 1) * 128],
                    lhsT=eblk[:, ebase + (KMAIN + r) * 128:
                              ebase + (KMAIN + r + 1) * 128],
                    rhs=oh[:], start=False, stop=(r == KREM - 1))
        if only in ("agg", "s1"):
            continue

        aggrT = spool.tile([P, SB * 128], f16, tag="aggrT", name="aggrT")
        nc.scalar.copy(aggrT[:], pag[:])

        # MLP (fp16 weights, fp32 PSUM accumulate)
        ph1 = psmlp.tile([P, SB * 128], f32, tag="mlp", name="mlp")
        nc.tensor.matmul(out=ph1[:], lhsT=consts["w0a"][:],
                         rhs=natT_s[:, s * SB * 128:(s + 1) * SB * 128],
                         start=True, stop=False)
        nc.tensor.matmul(out=ph1[:], lhsT=consts["w0b"][:], rhs=aggrT[:],
                         start=False, stop=True)
        h1 = spool.tile([P, SB * 128], f16, tag="h1", name="h1")
        nc.scalar.activation(h1[:], ph1[:], Act.Relu, bias=consts["b0"][:])

        ph2 = psmlp.tile([P, SB * 128], f32, tag="mlp", name="mlp")
        nc.tensor.matmul(out=ph2[:], lhsT=consts["w1"][:], rhs=h1[:],
                         start=True, stop=True)
        h2 = spool.tile([P, SB * 128], f16, tag="h2", name="h2")
        nc.scalar.activation(h2[:], ph2[:], Act.Relu, bias=consts["b1"][:])

        ph3 = psmlp.tile([P, SB * 128], f32, tag="mlp", name="mlp")
        nc.tensor.matmul(out=ph3[:], lhsT=consts["w2"][:], rhs=h2[:],
                         start=True, stop=True)
        h3T = spool.tile([P, SB * 128], f16, tag="h3T", name="h3T")
        nc.scalar.activation(h3T[:], ph3[:], Act.Identity,
                             bias=consts["b2"][:])
        sq = spool.tile([P, SB * 128], f16, tag="sq", name="sq")
        if TUNE["sq_engine"] == "dve":
            nc.vector.tensor_tensor(out=sq[:], in0=h3T[:], in1=h3T[:],
                                    op=Alu.mult)
        else:
            nc.scalar.activation(sq[:], h3T[:], Act.Square)

        # column stats: mu and E[x^2] per node into one aux PSUM bank
        paux = psaux.tile([P, 2 * SB], f32, tag="aux", name="aux")
        for b4 in range(SB):
            nc.tensor.matmul(out=paux[:, b4:b4 + 1],
                             lhsT=h3T[:, b4 * 128:(b4 + 1) * 128],
                             rhs=onesc[:], start=True, stop=True)
        for b4 in range(SB):
            nc.tensor.matmul(out=paux[:, SB + b4:SB + b4 + 1],
                             lhsT=sq[:, b4 * 128:(b4 + 1) * 128],
                             rhs=onesc[:], start=True, stop=True)
        mu_sb = spool.tile([P, SB], f32, tag="mu", name="mu")
        nc.scalar.copy(mu_sb[:], paux[:, 0:SB])
        musq = spool.tile([P, SB], f32, tag="musq", name="musq")
        nc.scalar.activation(musq[:], paux[:, 0:SB], Act.Square)
        var = spool.tile([P, SB], f32, tag="var", name="var")
        nc.vector.tensor_tensor(out=var[:], in0=paux[:, SB:2 * SB],
                                in1=musq[:], op=Alu.subtract)
        std = spool.tile([P, SB], f32, tag="std", name="std")
        nc.scalar.activation(std[:], var[:], Act.Sqrt, bias=epst[:])
        rstd = spool.tile([P, SB], f32, tag="rstd", name="rstd")
        nc.vector.reciprocal(rstd[:], std[:])

        pyt = psag.tile([P, SB, 128], f32, tag="py", name="py", bufs=2)
        for b4 in range(SB):
            nc.tensor.matmul(out=pyt[:, b4, :],
                             lhsT=h3T[:, b4 * 128:(b4 + 1) * 128],
                             rhs=consts["ident"][:], start=True, stop=True)
        xn = spool.tile([P, SB, 128], f16, tag="xn", name="xn")
        for b4 in range(SB):
            nc.vector.tensor_scalar(
                out=xn[:, b4, :], in0=pyt[:, b4, :],
                scalar1=mu_sb[:, b4:b4 + 1], scalar2=rstd[:, b4:b4 + 1],
                op0=Alu.subtract, op1=Alu.mult)
        geng = nc.gpsimd if TUNE["gam_engine"] == "gp" else nc.vector
        beng = nc.gpsimd if TUNE["beta_engine"] == "gp" else nc.vector
        yg = spool.tile([P, SB, 128], f16, tag="yg", name="yg")
        geng.tensor_tensor(out=yg[:], in0=xn[:], in1=consts["gam"][:],
                           op=Alu.mult)
        yo = spool.tile([P, SB, 128], f16, tag="yo", name="yo")
        beng.tensor_tensor(out=yo[:], in0=yg[:], in1=consts["bet"][:],
                           op=Alu.add)
        odma.dma_start(out=out[s], in_=yo[:])


def _prepare_shards(node_attr, edge_attr, col):
    """Fixed 16-slot-per-node main region + LPT on overflow for remainder."""
    import heapq

    deg = np.bincount(col, minlength=NUM_NODES).astype(np.int64)
    over = np.maximum(deg - CAP, 0)
    order_nodes = np.argsort(-over, kind="stable")
    heap = [(0, 0, b) for b in range(TOTAL_BLOCKS)]
    heapq.heapify(heap)
    block_nodes = [[] for _ in range(TOTAL_BLOCKS)]
    for nd in order_nodes:
        d = int(over[nd])
        s, cnt, b = heapq.heappop(heap)
        block_nodes[b].append(int(nd))
        if cnt + 1 < BLK:
            heapq.heappush(heap, (s + d, cnt + 1, b))
    rem_max = max(sum(int(over[nd]) for nd in bn) for bn in block_nodes)
    assert rem_max <= KREM * 128, rem_max

    pos = np.full(NUM_NODES, -1, dtype=np.int64)      # old -> new node id
    natp = np.zeros((TOTAL_BLOCKS * BLK, D), np.float16)
    for b, bn in enumerate(block_nodes):
        ids = np.asarray(bn, dtype=np.int64)
        pos[ids] = b * BLK + np.arange(len(ids))
        natp[b * BLK:b * BLK + len(ids)] = node_attr[ids].astype(np.float16)
    assert (pos >= 0).all()

    # per-edge slot assignment
    order = np.argsort(col, kind="stable")           # edges grouped per node
    cs = col[order]
    within = np.arange(col.shape[0], dtype=np.int64)
    starts = np.zeros(NUM_NODES + 1, np.int64)
    starts[1:] = np.cumsum(deg)
    within = within - starts[cs]                     # rank within node
    npos = pos[cs]
    blk = npos >> 7
    loc = npos & 127

    main_mask = within < CAP
    slot = np.empty(col.shape[0], dtype=np.int64)
    slot[main_mask] = (blk[main_mask] * KTOT * 128 + loc[main_mask] * CAP +
                       within[main_mask])
    # overflow edges: sequential within their block's remainder region
    om = ~main_mask
    oblk = blk[om]
    oord = np.argsort(oblk, kind="stable")
    ocnt = np.bincount(oblk, minlength=TOTAL_BLOCKS)
    ostart = np.zeros(TOTAL_BLOCKS + 1, np.int64)
    ostart[1:] = np.cumsum(ocnt)
    opos_in_blk = np.arange(om.sum(), dtype=np.int64) - ostart[oblk[oord]]
    oslot = np.empty(om.sum(), dtype=np.int64)
    oslot[oord] = (oblk[oord] * KTOT * 128 + KMAIN * 128 + opos_in_blk)
    slot[om] = oslot

    ea16 = edge_attr.astype(np.float16)
    slots_per_core = BLOCKS_PER_CORE * KTOT * 128
    edges_by_core = []
    colf_by_core = []
    natT_by_core = []
    blk_of = slot // (KTOT * 128)
    off_of = slot % (KTOT * 128)
    loc_f = loc.astype(np.float32)
    for c in range(N_CORES):
        sel = (blk_of >= c * BLOCKS_PER_CORE) & \
              (blk_of < (c + 1) * BLOCKS_PER_CORE)
        lblk = blk_of[sel] - c * BLOCKS_PER_CORE
        lslot = lblk * (KTOT * 128) + off_of[sel]
        ebuf = np.zeros((slots_per_core, D), np.float16)
        ebuf[lslot] = ea16[order[sel]]
        earr = np.ascontiguousarray(
            ebuf.reshape(BLOCKS_PER_CORE * KTOT, 128, D)
            .transpose(1, 0, 2).reshape(P, slots_per_core))
        edges_by_core.append(earr)
        cf = np.full((BLOCKS_PER_CORE, KREM, 128), -1.0, np.float32)
        rm = off_of[sel] >= KMAIN * 128
        roff = off_of[sel][rm] - KMAIN * 128
        cf[lblk[rm], roff // 128, roff % 128] = loc_f[sel][rm]
        carr = np.ascontiguousarray(
            cf.reshape(BLOCKS_PER_CORE * KREM, 128).T)
        colf_by_core.append(carr)
        natT_by_core.append(np.ascontiguousarray(
            natp[c * NODES_PER_CORE:(c + 1) * NODES_PER_CORE].T))
    kb = (KTOT,) * BLOCKS_PER_CORE
    return kb, edges_by_core, colf_by_core, natT_by_core, pos


def kernel(node_attr, edge_attr, edge_index, W0, b0, W1, b1, W2, b2,
           ln_g, ln_b):
    from concourse import bass_utils

    node_attr = np.ascontiguousarray(np.asarray(node_attr, dtype=np.float32))
    edge_attr = np.ascontiguousarray(np.asarray(edge_attr, dtype=np.float32))
    col = np.asarray(edge_index)[1].astype(np.int64)
    W0 = np.asarray(W0, dtype=np.float32)
    W1 = np.asarray(W1, dtype=np.float32)
    W2 = np.asarray(W2, dtype=np.float32)
    b0v = np.asarray(b0, np.float32).reshape(128, 1).copy()
    b1v = np.asarray(b1, np.float32).reshape(128, 1).copy()
    b2v = np.asarray(b2, np.float32).reshape(128, 1).copy()
    gam = np.ascontiguousarray(
        np.broadcast_to(np.asarray(ln_g, np.float32).reshape(1, 1, 128),
                        (128, SB, 128)).astype(np.float16))
    bet = np.ascontiguousarray(
        np.broadcast_to(np.asarray(ln_b, np.float32).reshape(1, 1, 128),
                        (128, SB, 128)).astype(np.float16))

    kb, edges_by_core, colf_by_core, natT_by_core, pos = _prepare_shards(
        node_attr, edge_attr, col)

    iota_rep = np.ascontiguousarray(
        np.broadcast_to(np.arange(128, dtype=np.float16), (P, 128)))
    rmat = np.zeros((128, 32), np.float16)
    rmat[np.arange(128), np.arange(128) // G] = 1.0
    # S_q[p, j] = 1 iff node j owns group at partition p of quad-column q:
    #   j = 32 q + 8 (p//32) + (p%32)//4
    smat = np.zeros((128, SB, 128), np.float16)
    pidx = np.arange(128)
    for q in range(SB):
        smat[pidx, q, 32 * q + 8 * (pidx // 32) + (pidx % 32) // 4] = 1.0
    ident = np.eye(128, dtype=np.float16)

    if kb not in _nc_cache:
        _nc_cache[kb] = _build_nc(kb)
    nc = _nc_cache[kb]

    shared = {"iota": iota_rep, "rmat": rmat, "smat": smat, "ident": ident,
              "w0a": np.ascontiguousarray(W0[:128].astype(np.float16)),
              "w0b": np.ascontiguousarray(W0[128:].astype(np.float16)),
              "w1": np.ascontiguousarray(W1.astype(np.float16)),
              "w2": np.ascontiguousarray(W2.astype(np.float16)),
              "gam": gam, "bet": bet, "b0": b0v, "b1": b1v, "b2": b2v}
    in_maps = []
    for c in range(N_CORES):
        m = {"edges": edges_by_core[c], "colf32": colf_by_core[c],
             "natT": natT_by_core[c]}
        m.update(shared)
        in_maps.append(m)

    res = bass_utils.run_bass_kernel_spmd(nc, in_maps,
                                          core_ids=list(range(N_CORES)))
    last_run_info["results"] = res
    last_run_info["nc"] = nc
    last_run_info["in_maps"] = in_maps
    last_run_info["kb"] = kb
    last_run_info["pos"] = pos

    rows = np.concatenate(
        [res.results[c]["out"].reshape(SBLOCKS, P, SB, 128)
         .transpose(0, 2, 1, 3).reshape(NODES_PER_CORE, D)
         for c in range(N_CORES)], axis=0)
    return rows[pos].astype(np.float32)


if __name__ == "__main__":
    pass
